# revision 23
# baseline (speedup 1.0000x reference)
"""Two-layer GAT (GraphAttention) forward on 8 Trainium2 NeuronCores.

Math (per layer, reference semantics):
    h  = x @ w                      [N, U]
    a1 = h @ aw1 ; a2 = h @ aw2     [N, H]
    P[i,j,h]    = exp(adj[i,j] * relu(a1[i,h] + a2[j,h]))
    attn[i,j,h] = P / sum_j P
    out[i,h,:]  = sum_j attn[i,j,h] * h[j,:]   -> concat heads -> activation

Key identity used here: with adj in {0,1},
    P[i,j] = max(adj[i,j] * e1[i] * e2[j], 1),   e1 = exp(a1), e2 = exp(a2)
and softmax rows are invariant to scaling by em1[i] = exp(-a1[i]):
    P'[j,i] = max(adjT[j,i] * e2[j], em1[i])
so the per-element work is one tensor_scalar (4x DVE mode, per-partition e2)
plus one tensor_tensor max (2x DVE mode) against a broadcast em1 row, with
numerator and denominator both coming out of a single PE matmul against
[h | 1] extended features.

Sharding: rows (i) of the score matrix are partitioned 512 per core;
adj rows are fed pre-transposed per core as [N, 512] (j on partitions).
All small weights are replicated; hfeat of layer 1 is all-gathered.
"""

import sys

for _p in ("/opt/trn_rl_repo",):
    if _p not in sys.path:
        sys.path.insert(0, _p)

from contextlib import ExitStack

import ml_dtypes
import numpy as np

import concourse.bacc as bacc
import concourse.mybir as mybir
import concourse.tile as tile
from concourse.bass_utils import run_bass_kernel_spmd

F32 = mybir.dt.float32
BF16 = mybir.dt.bfloat16
BF = ml_dtypes.bfloat16

N = 4096          # nodes
FIN = 128         # input features
U0 = 16           # layer-0 units
H0 = 4            # layer-0 heads
NCORES = 8
R = N // NCORES   # local rows per core (512)
NJT = N // 128    # j tiles (32)
GRP = 8           # j-tiles fused per tensor_tensor max
NGRP = NJT // GRP

# Of each group of 8 B-tiles (B = adjT * e2[j]), how many go to DVE
# (tensor_scalar, 4x mode) vs ACT (activation Copy with per-partition scale).
DVE_B_PER_GRP = 4

_CACHE = {}
DEBUG = False


def _build():
    nc = bacc.Bacc("TRN2", target_bir_lowering=False, debug=False,
                   num_devices=NCORES)

    # ---- I/O ----
    d_adjT = nc.dram_tensor("adjT", [N, R], BF16, kind="ExternalInput")
    d_xT = nc.dram_tensor("xT", [FIN, N], BF16, kind="ExternalInput")
    d_xTl = nc.dram_tensor("xTl", [FIN, R], BF16, kind="ExternalInput")
    d_prep = nc.dram_tensor("prep", [FIN, U0 + H0], BF16, kind="ExternalInput")
    d_v1 = nc.dram_tensor("v1", [FIN, H0], BF16, kind="ExternalInput")
    d_w1e = nc.dram_tensor("w1e", [(U0 + 1) * H0, 1], F32, kind="ExternalInput")
    d_aw11 = nc.dram_tensor("aw11", [1, 1], F32, kind="ExternalInput")
    d_aw21 = nc.dram_tensor("aw21", [1, 1], F32, kind="ExternalInput")
    d_y = nc.dram_tensor("y", [1, R], F32, kind="ExternalOutput")
    dbg = {}
    if DEBUG:
        for nm, shp, dt in [("em1", [H0, R], BF16), ("e2j", [128, NJT * H0], F32),
                            ("hj", [128, NJT * (U0 + 1)], BF16),
                            ("den", [H0, R], F32), ("h1T", [U0 * H0, R], F32),
                            ("hfeT", [1, R], F32), ("hfe1", [128, NJT], F32),
                            ("em1bc0", [128, R], BF16), ("acc0", [U0 + 1, R], F32),
                            ("e2j1", [128, NJT], F32), ("em1bc1", [128, R], BF16),
                            ("l1acc", [2, R], F32),
                            ("h1raw", [U0 * H0, R], F32), ("recbc", [U0 * H0, R], F32)]:
            dbg[nm] = nc.dram_tensor("dbg_" + nm, shp, dt, kind="ExternalOutput")

    with ExitStack() as ctx:
        tc = ctx.enter_context(tile.TileContext(nc))
        const = ctx.enter_context(tc.tile_pool(name="const", bufs=1))
        work = ctx.enter_context(tc.tile_pool(name="work", bufs=1))
        bpool = ctx.enter_context(tc.tile_pool(name="bpool", bufs=4))
        ppool = ctx.enter_context(tc.tile_pool(name="ppool", bufs=4))
        accs = ctx.enter_context(tc.tile_pool(name="accs", bufs=2))
        dram = ctx.enter_context(tc.tile_pool(name="dram", bufs=1, space="DRAM"))
        pp_misc = ctx.enter_context(tc.tile_pool(name="pp_misc", bufs=2, space="PSUM"))
        pp_hj = ctx.enter_context(tc.tile_pool(name="pp_hj", bufs=2, space="PSUM"))
        pp_acc = ctx.enter_context(tc.tile_pool(name="pp_acc", bufs=2, space="PSUM"))

        # ---- persistent SBUF ----
        sb_adjT = const.tile([128, NJT * R], BF16, tag="adjT")     # 32KB/p
        sb_xT = const.tile([FIN, N], BF16, tag="xT")               # 8KB/p
        sb_xTl = const.tile([FIN, R], BF16, tag="xTl")
        sb_prep = const.tile([FIN, U0 + H0], BF16, tag="prep")     # [w0 | v2]
        sb_v1 = const.tile([FIN, H0], BF16, tag="v1")
        sb_aw11 = const.tile([1, 1], F32, tag="aw11")
        sb_naw11 = const.tile([1, 1], F32, tag="naw11")
        sb_aw21bc = const.tile([128, 1], F32, tag="aw21bc")
        sb_hj = const.tile([128, NJT * (U0 + 1)], BF16, tag="hj")  # [h | 1] per jt
        sb_e2j = const.tile([128, NJT * H0], F32, tag="e2j")
        sb_em1bc = [const.tile([128, R], BF16, tag=f"em1bc{h}",
                       name=f"em1bc{h}") for h in range(H0)]
        sb_rec = [const.tile([1, R], F32, tag=f"rec{h}", name=f"rec{h}")
                  for h in range(H0)]
        sb_h1raw = [const.tile([U0 + 1, R], F32, tag=f"h1raw{h}", name=f"h1raw{h}")
                    for h in range(H0)]
        sb_w1h = [const.tile([U0 + 1, 1], F32, tag=f"w1h{h}", name=f"w1h{h}")
                  for h in range(H0)]
        sb_hfp = [const.tile([1, R], F32, tag=f"hfp{h}", name=f"hfp{h}")
                  for h in range(H0)]
        sb_em1s = [const.tile([1, R], BF16, tag=f"em1s{h}", name=f"em1s{h}")
                   for h in range(H0)]
        sb_hfeT = const.tile([1, R], F32, tag="hfeT")
        sb_em11 = const.tile([1, R], BF16, tag="em11")
        sb_em1bc1 = const.tile([128, R], BF16, tag="em1bc1")
        sb_hfe1 = const.tile([128, NJT], F32, tag="hfe1")
        sb_e2j1 = const.tile([128, NJT], F32, tag="e2j1")
        sb_hfe1e = const.tile([128, NJT * 2], BF16, tag="hfe1e")
        sb_sigd = work.tile([1, 1], F32, tag="sigd")
        sb_fin = work.tile([1, R], F32, tag="fin")
        sb_fin2 = work.tile([1, R], F32, tag="fin2")

        d_em1 = dram.tile([H0, R], BF16)  # per-head rows
        d_em11 = dram.tile([1, R], BF16)
        d_gin = dram.tile([1, R], F32)
        d_gout = dram.tile([NCORES, R], F32, addr_space="Shared")

        # ---- load constants / inputs ----
        nc.sync.dma_start(sb_prep[:], d_prep[:])
        nc.sync.dma_start(sb_v1[:], d_v1[:])
        nc.sync.dma_start(sb_xTl[:], d_xTl[:])
        for h in range(H0):
            nc.sync.dma_start(sb_w1h[h][:],
                              d_w1e[(U0 + 1) * h:(U0 + 1) * (h + 1), :])
        nc.sync.dma_start(sb_aw11[:], d_aw11[:])
        nc.sync.dma_start(sb_aw21bc[:], d_aw21[0:1, 0:1].to_broadcast((128, 1)))
        for xc in range(4):
            nc.scalar.dma_start(sb_xT[:, 1024 * xc:1024 * (xc + 1)],
                                d_xT[:, 1024 * xc:1024 * (xc + 1)])
        # adjT: 2 j-tiles per DMA for 2KB per-partition lines
        for m in range(NJT // 2):
            src = d_adjT[256 * m:256 * (m + 1), :].rearrange(
                "(g p) i -> p g i", p=128)
            dst = sb_adjT[:, 1024 * m:1024 * (m + 1)].rearrange(
                "p (g i) -> p g i", g=2)
            nc.scalar.dma_start(dst, src)

        for h in range(H0):
            ps_a1 = pp_misc.tile([1, R], F32, tag="misc", name="ps_a1")
            nc.tensor.matmul(ps_a1[:], sb_v1[:, h:h + 1], sb_xTl[:],
                             start=True, stop=True)
            nc.scalar.activation(sb_em1s[h][:], ps_a1[:],
                                 mybir.ActivationFunctionType.Exp, scale=-1.0)
            nc.sync.dma_start(d_em1[h:h + 1, :], sb_em1s[h][:])
            nc.sync.dma_start(sb_em1bc[h][:],
                              d_em1[h:h + 1, :].to_broadcast((128, R)))

        # ---- prep: h/e2 per j-tile ----
        nc.vector.memset(sb_hj[:], 1.0)
        W = U0 + H0
        for q4 in range(NJT // 4):
            ps4 = pp_hj.tile([128, 4 * W], F32, tag="hj", name="ps4")
            for q in range(4):
                jt = 4 * q4 + q
                nc.tensor.matmul(ps4[:, W * q:W * (q + 1)],
                                 sb_xT[:, 128 * jt:128 * (jt + 1)],
                                 sb_prep[:], start=True, stop=True)
            hjv = sb_hj[:, 4 * (U0 + 1) * q4:4 * (U0 + 1) * (q4 + 1)].rearrange(
                "p (q c) -> p q c", q=4)[:, :, 1:U0 + 1]
            psv = ps4[:].rearrange("p (q c) -> p q c", q=4)[:, :, 0:U0]
            nc.vector.tensor_copy(hjv, psv)
            e2v = sb_e2j[:, 4 * H0 * q4:4 * H0 * (q4 + 1)].rearrange(
                "p (q c) -> p q c", q=4)
            pse = ps4[:].rearrange("p (q c) -> p q c", q=4)[:, :, U0:U0 + H0]
            nc.scalar.activation(e2v, pse, mybir.ActivationFunctionType.Exp)

        nc.vector.memset(sb_hfe1e[:], 1.0)

        # ---- layer 0 main ----
        for h in range(H0):
            ps_acc = pp_acc.tile([U0 + 1, R], F32, tag="acc")
            for g in range(NGRP):
                t_B = bpool.tile([128, GRP * R], BF16, tag="B")
                for k in list(range(DVE_B_PER_GRP, GRP)) + list(range(DVE_B_PER_GRP)):
                    jt = GRP * g + k
                    dst = t_B[:, R * k:R * (k + 1)]
                    src = sb_adjT[:, R * jt:R * (jt + 1)]
                    sc = sb_e2j[:, H0 * jt + h:H0 * jt + h + 1]
                    if k < DVE_B_PER_GRP:
                        nc.vector.tensor_scalar_mul(dst, src, sc)
                    else:
                        nc.scalar.mul(dst, src, sc)
                t_P = ppool.tile([128, GRP * R], BF16, tag="P")
                nc.vector.tensor_tensor(
                    t_P[:].rearrange("p (g i) -> p g i", g=GRP),
                    t_B[:].rearrange("p (g i) -> p g i", g=GRP),
                    sb_em1bc[h][:, None, :].to_broadcast((128, GRP, R)),
                    mybir.AluOpType.max)
                for k in range(GRP):
                    jt = GRP * g + k
                    nc.tensor.matmul(
                        ps_acc[:],
                        sb_hj[:, (U0 + 1) * jt:(U0 + 1) * (jt + 1)],
                        t_P[:, R * k:R * (k + 1)],
                        start=(jt == 0), stop=(jt == NJT - 1))
            nc.scalar.activation(sb_h1raw[h][:], ps_acc[:],
                                 mybir.ActivationFunctionType.Relu)
            nc.vector.reciprocal_approx_accurate(
                sb_rec[h][:], sb_h1raw[h][0:1, :],
                accs.tile([1, R], F32, tag="rscr", name="rscr"))
            ps_s = pp_misc.tile([1, R], F32, tag="misc", name="ps_s")
            nc.tensor.matmul(ps_s[:], sb_w1h[h][:], sb_h1raw[h][:],
                             start=True, stop=True)
            nc.vector.tensor_mul(sb_hfp[h][:], ps_s[:], sb_rec[h][:])
            if h == 1:
                nc.vector.tensor_add(sb_hfp[0][:], sb_hfp[0][:], sb_hfp[1][:])
            elif h == 2:
                nc.vector.tensor_add(sb_hfp[2][:], sb_hfp[2][:], sb_hfp[0][:])


        # ---- layer 1 prep: hfeT = sum_h rec_h * (w1_h^T @ h1raw_h) ----
        nc.vector.tensor_add(sb_hfeT[:], sb_hfp[2][:], sb_hfp[3][:])
        nc.scalar.mul(sb_naw11[:], sb_aw11[:], -1.0)
        nc.scalar.activation(sb_em11[:], sb_hfeT[:],
                             mybir.ActivationFunctionType.Exp,
                             scale=sb_naw11[:])
        nc.sync.dma_start(d_em11[:], sb_em11[:])
        nc.sync.dma_start(sb_em1bc1[:], d_em11[0:1, :].to_broadcast((128, R)))
        nc.sync.dma_start(d_gin[:], sb_hfeT[:])
        nc.gpsimd.collective_compute(
            "AllGather", mybir.AluOpType.bypass,
            replica_groups=[list(range(NCORES))],
            ins=[d_gin[:].opt()], outs=[d_gout[:].opt()])
        gflat = d_gout[:].rearrange("a b -> (a b)").rearrange(
            "(t p) -> p t", p=128)
        nc.gpsimd.dma_start(sb_hfe1[:], gflat)
        nc.scalar.activation(sb_e2j1[:], sb_hfe1[:],
                             mybir.ActivationFunctionType.Exp,
                             scale=sb_aw21bc[:])
        nc.scalar.activation(sb_sigd[:], sb_sigd[:],
                             mybir.ActivationFunctionType.Sigmoid)
        nc.vector.tensor_copy(
            sb_hfe1e[:].rearrange("p (t two) -> p t two", two=2)[:, :, 0:1],
            sb_hfe1[:][:, :, None])

        # ---- layer 1 main ----
        ps_l1n = pp_acc.tile([1, R], F32, tag="accn", name="ps_l1n", bufs=1)
        ps_l1d = pp_acc.tile([1, R], F32, tag="accd", name="ps_l1d", bufs=1)
        for g in range(NGRP):
            t_B = bpool.tile([128, GRP * R], BF16, tag="B")
            for k in list(range(DVE_B_PER_GRP, GRP)) + list(range(DVE_B_PER_GRP)):
                jt = GRP * g + k
                dst = t_B[:, R * k:R * (k + 1)]
                src = sb_adjT[:, R * jt:R * (jt + 1)]
                sc = sb_e2j1[:, jt:jt + 1]
                if k < DVE_B_PER_GRP:
                    nc.vector.tensor_scalar_mul(dst, src, sc)
                else:
                    nc.scalar.mul(dst, src, sc)
            t_P = ppool.tile([128, GRP * R], BF16, tag="P")
            nc.vector.tensor_tensor(
                t_P[:].rearrange("p (g i) -> p g i", g=GRP),
                t_B[:].rearrange("p (g i) -> p g i", g=GRP),
                sb_em1bc1[:, None, :].to_broadcast((128, GRP, R)),
                mybir.AluOpType.max)
            for k in range(GRP):
                jt = GRP * g + k
                nc.tensor.matmul(
                    ps_l1n[:], sb_hfe1e[:, 2 * jt:2 * jt + 1],
                    t_P[:, R * k:R * (k + 1)],
                    start=(jt == 0), stop=(jt == NJT - 1))
                nc.tensor.matmul(
                    ps_l1d[:], sb_hfe1e[:, 2 * jt + 1:2 * jt + 2],
                    t_P[:, R * k:R * (k + 1)],
                    start=(jt == 0), stop=(jt == NJT - 1))

        # ---- final: sigmoid(numer/denom) ----
        sb_fscr = accs.tile([1, R], F32, tag="fscr", name="sb_fscr")
        nc.vector.reciprocal_approx_accurate(sb_fin[:], ps_l1d[:], sb_fscr[:])
        nc.vector.tensor_mul(sb_fin2[:], ps_l1n[:], sb_fin[:])
        nc.scalar.activation(sb_fin[:], sb_fin2[:],
                             mybir.ActivationFunctionType.Sigmoid)
        nc.sync.dma_start(d_y[:], sb_fin[:])
        if DEBUG:
            nc.sync.dma_start(dbg["e2j"][:], sb_e2j[:])
            nc.sync.dma_start(dbg["hj"][:], sb_hj[:])
            nc.sync.dma_start(dbg["hfeT"][:], sb_hfeT[:])
            nc.sync.dma_start(dbg["hfe1"][:], sb_hfe1[:])
            nc.sync.dma_start(dbg["em1bc0"][:], sb_em1bc[0][:])
            nc.sync.dma_start(dbg["e2j1"][:], sb_e2j1[:])
            nc.sync.dma_start(dbg["em1bc1"][:], sb_em1bc1[:])

    nc.compile()
    return nc


def _prep_inputs(x, adj, w0, aw1_0, aw2_0, w1, aw1_1, aw2_1):
    x = np.asarray(x, np.float32)
    adj = np.asarray(adj, np.float32)
    xT = np.ascontiguousarray(x.T.astype(BF))
    adjT = np.asarray(adj.T, BF)                        # [N, N], exact 0/1
    w0f = np.asarray(w0, np.float32)
    v1 = np.ascontiguousarray((w0f @ np.asarray(aw1_0, np.float32)).astype(BF))
    v2 = (w0f @ np.asarray(aw2_0, np.float32)).astype(BF)
    prep = np.ascontiguousarray(
        np.concatenate([w0f.astype(BF), v2], axis=1))
    w1f = np.asarray(w1, np.float32).reshape(H0, U0)
    w1e = np.zeros((H0, U0 + 1), np.float32)
    w1e[:, 1:] = w1f
    w1e = np.ascontiguousarray(w1e.reshape((U0 + 1) * H0, 1))
    aw11 = np.asarray(aw1_1, np.float32).reshape(1, 1)
    aw21 = np.asarray(aw2_1, np.float32).reshape(1, 1)
    in_maps = []
    for c in range(NCORES):
        rows = slice(R * c, R * (c + 1))
        in_maps.append({
            "adjT": np.ascontiguousarray(adjT[:, rows]),
            "xT": xT,
            "xTl": np.ascontiguousarray(xT[:, rows]),
            "prep": prep, "v1": v1, "w1e": w1e,
            "aw11": aw11, "aw21": aw21,
        })
    return in_maps


def run(inputs, trace=False):
    if "nc" not in _CACHE:
        _CACHE["nc"] = _build()
    nc = _CACHE["nc"]
    in_maps = _prep_inputs(**inputs)
    res = run_bass_kernel_spmd(nc, in_maps, list(range(NCORES)), trace=trace)
    y = np.concatenate([res.results[c]["y"][0] for c in range(NCORES)])
    return np.ascontiguousarray(y.astype(np.float32)), res


def kernel(**inputs):
    y, _ = run(inputs)
    return y
